# revision 63
# baseline (speedup 1.0000x reference)
"""Distributed Trainium2 kernel for nn_Attn_77970836292156.

Cross-attention block: fused QKV projection + per-head RMSNorm + RoPE +
bf16 SDPA (4096 keys = 2048 self + 2048 cross) + output projection.

Sharding: tensor-parallel on heads. 16 heads / 8 cores = 2 heads per core.
W_qkv / W_ckv column-sharded by head; every core holds full x, y (transposed,
bf16). Attention runs fully local per core in a transposed layout
(head-dims on partitions, positions on the free axis), producing
OT [128 dims, 2048 q]. An AllToAll converts head-sharding -> sequence-
sharding, then each core applies the full W_out to its 256-row slice
(row-sharded matmul accumulated over all 1024 dims), so no AllReduce is
needed and the output projection's reduction happens on the TensorEngine.

Changes vs the original baseline:
- Softmax exp split across engines: head 0 on ACT (table exp), head 1 on
  DVE via a Schraudolph bit-trick (bits = trunc(score*a + b) as int16,
  reinterpreted bf16) -- halves the softmax bottleneck.
- Attention inner loop software-pipelined: the two heads' QK matmuls are
  emitted adjacently (they occupy different PE row groups and execute
  concurrently); each kc's PV matmuls are deferred one iteration so they
  fill the in-order PE queue while the exps run.
- RMSNorm rsqrt batched into ONE Ln + ONE Exp over all 3 projections
  (kills ~19 ACT table reloads); per-head broadcast via a tiny
  stationary-selector matmul on the PE instead of DRAM bounces.
- Softmax denominators: reciprocal via ACT Ln/Exp(-x) (the DVE iterative
  reciprocal was 6.4us per call), broadcast on the idle GPSIMD.
- V projection computed transposed (512-wide matmuls) then PE-transposed
  back, instead of 256 tiny 128-free matmuls.
- Out-projections emitted after both attention halves: qh0's executes
  during qh1's AllToAll window; only qh1's is exposed in the tail.
- Input loads split into position chunks so compute starts early; W_out
  load deferred into phase 2 so startup DMA bandwidth goes to x/y.
"""

import os

import numpy as np
import ml_dtypes

import concourse.bass as bass
import concourse.tile as tile
from concourse import bacc, mybir
from concourse.bass_utils import run_bass_kernel_spmd

BF16 = mybir.dt.bfloat16
F32 = mybir.dt.float32
I16 = mybir.dt.int16

# Problem constants (hardcoded per spec).
N = 2048        # query positions
M = 2048        # cross positions
NK = N + M      # total keys
D = 1024        # model dim
H = 16          # heads
DH = 64         # head dim
HL = 2          # heads per core
DL = HL * DH    # local head dims = 128
F = 1024        # input features
P = 128
NCORES = 8
EPS = 1e-6
ROPE_BASE = 10000.0
SCALE = 0.125   # 1/sqrt(64)

# Schraudolph exp constants for bf16 bits = trunc(score*EXA + EXB):
#   bits = 128*(score*SCALE*log2 e) + 127*128 - 5.5 (minimax centering)
#   + 0.5 (truncation compensation)
EXA = SCALE * 128.0 * 1.4426950408889634
EXB = 16251.0

LAST_RESULT = None  # test harness reads exec_time_ns from here


def build_nc():
    nc = bacc.Bacc()

    # ---------------- DRAM parameters ----------------
    # x/y arrive host-prearranged chunk-major [p, chunk, f, 512] so each
    # position-chunk load is one contiguous 8KB run per partition.
    xT = nc.declare_dram_parameter("xT", [P, 4, 8, 512], BF16, isOutput=False)
    yT = nc.declare_dram_parameter("yT", [P, 4, 8, 512], BF16, isOutput=False)
    # weights arrive host-prearranged as [p, f, c] so each partition's DMA
    # is one contiguous 2KB run (8x fewer, 8x bigger descriptors).
    wq = nc.declare_dram_parameter("wq", [P, 8, DL], BF16, isOutput=False)
    wk = nc.declare_dram_parameter("wk", [P, 8, DL], BF16, isOutput=False)
    wv = nc.declare_dram_parameter("wv", [P, 8, DL], BF16, isOutput=False)
    wck = nc.declare_dram_parameter("wck", [P, 8, DL], BF16, isOutput=False)
    wcv = nc.declare_dram_parameter("wcv", [P, 8, DL], BF16, isOutput=False)
    wo = nc.declare_dram_parameter("wo", [P, 8, D], BF16, isOutput=False)
    bo = nc.declare_dram_parameter("bo", [1, D], BF16, isOutput=False)
    cq = nc.declare_dram_parameter("cq", [P, N], BF16, isOutput=False)
    sq = nc.declare_dram_parameter("sq", [P, N], BF16, isOutput=False)
    ckc = nc.declare_dram_parameter("ckc", [P, NK], BF16, isOutput=False)
    cks = nc.declare_dram_parameter("cks", [P, NK], BF16, isOutput=False)
    hmask = nc.declare_dram_parameter("hmask", [P, HL], BF16, isOutput=False)
    hsel = nc.declare_dram_parameter("hsel", [HL, P], BF16, isOutput=False)
    ident = nc.declare_dram_parameter("ident", [P, P], BF16, isOutput=False)
    out_ext = nc.declare_dram_parameter("out", [N // NCORES, D], F32, isOutput=True)

    # A2A bounce buffers (collectives can't touch I/O tensors).
    a2a_in = nc.dram_tensor("a2a_in", [2, NCORES, P, P], BF16)
    a2a_out = nc.dram_tensor("a2a_out", [2, NCORES, P, P], BF16)

    with tile.TileContext(nc) as tc, \
            tc.tile_pool(name="singles", bufs=1) as singles:

        # ---------------- static SBUF loads ----------------
        def load_w(param):
            t = singles.tile([P, 8, DL], BF16, tag=param.name + "_sb")
            nc.sync.dma_start(out=t, in_=param[:, :, :])
            return t

        wq_sb, wk_sb, wv_sb, wck_sb, wcv_sb = (
            load_w(w) for w in (wq, wk, wv, wck, wcv))

        hmask_sb = singles.tile([P, HL], BF16)
        nc.sync.dma_start(out=hmask_sb, in_=hmask[:, :])
        hsel_sb = singles.tile([HL, P], BF16)
        nc.sync.dma_start(out=hsel_sb, in_=hsel[:, :])
        ident_sb = singles.tile([P, P], BF16)
        nc.sync.dma_start(out=ident_sb, in_=ident[:, :])
        bo_sb = singles.tile([1, D], BF16)
        nc.sync.dma_start(out=bo_sb, in_=bo[0:1, :])
        # bias pre-broadcast across partitions: lets the out-projection add
        # it during eviction instead of two extra PE matmuls per half.
        bo_b = singles.tile([P, D], BF16)
        nc.gpsimd.partition_broadcast(bo_b[0:P, :], bo_sb[0:1, :], channels=P)

        ones1 = singles.tile([1, P], BF16)
        nc.vector.memset(ones1, 1.0)
        eps2 = singles.tile([HL, 1], F32)
        nc.vector.memset(eps2, EPS)

        # Normed/roped activations in transposed layout.
        qTn = singles.tile([P, N], BF16)
        kTn = singles.tile([P, NK], BF16)
        # V in natural layout [keys, dims], 130 = [h0 64 | 1 | h1 64 | 1].
        v_all = singles.tile([P, NK // P, 130], BF16)
        nc.gpsimd.memset(v_all, 1.0)

        # ---------------- phase 1: projections + RMSNorm + RoPE ----------------
        with tc.tile_pool(name="proj_ps", bufs=2, space="PSUM") as proj_ps, \
                tc.tile_pool(name="ssq_ps", bufs=2, space="PSUM") as ssq_ps, \
                tc.tile_pool(name="trps", bufs=2, space="PSUM") as trps, \
                tc.tile_pool(name="rsb_ps", bufs=2, space="PSUM") as rsb_ps, \
                tc.tile_pool(name="p1big", bufs=1) as p1big, \
                tc.tile_pool(name="rope", bufs=1) as rope, \
                tc.tile_pool(name="p1work", bufs=4) as p1work:

            # chunk-major [p, chunk, f, 512]: position-chunked loads (so the
            # first projections start early) are fully contiguous.
            xT_sb = p1big.tile([P, 4, 8, 512], BF16)
            yT_sb = p1big.tile([P, 4, 8, 512], BF16)
            for c4 in range(4):
                nc.sync.dma_start(out=xT_sb[:, c4], in_=xT[:, c4])
            cq_sb = p1big.tile([P, N], BF16)
            sq_sb = p1big.tile([P, N], BF16)
            nc.sync.dma_start(out=cq_sb, in_=cq[:, :])
            nc.sync.dma_start(out=sq_sb, in_=sq[:, :])
            ckc_sb = p1big.tile([P, NK], BF16)
            cks_sb = p1big.tile([P, NK], BF16)
            nc.sync.dma_start(out=ckc_sb, in_=ckc[:, :])
            nc.sync.dma_start(out=cks_sb, in_=cks[:, :])
            for c4 in range(4):
                nc.sync.dma_start(out=yT_sb[:, c4], in_=yT[:, c4])

            # mean-squares for all 3 projections land here ([2, 3*2048] f32)
            ssq_all = p1big.tile([HL, 3 * N], F32)

            def proj_chunks(w_sb, src_sb, dst, dst_off, ssq_off):
                """Projection matmuls + raw evict + squares + per-head
                mean-squares for one 2048-wide projection."""
                qsqs = []
                for t in range(4):
                    ps = proj_ps.tile([P, 512], F32, tag="proj")
                    for f in range(8):
                        nc.tensor.matmul(ps, w_sb[:, f, :],
                                         src_sb[:, t, f, :],
                                         start=(f == 0), stop=(f == 7))
                    # raw evict on ACT (Copy is in every table set)
                    raw = dst[:, dst_off + t * 512:dst_off + (t + 1) * 512]
                    nc.scalar.activation(
                        out=raw, in_=ps,
                        func=mybir.ActivationFunctionType.Copy)
                    # squares on DVE from the evicted bf16 (2x mode)
                    qsq = p1work.tile([P, 512], BF16, tag="qsq")
                    nc.vector.tensor_mul(qsq, raw, raw)
                    qsqs.append(qsq)
                # mean-squares last: by the time the PE reaches these, the
                # ACT/DVE chains above have drained (no in-order PE stall).
                for t in range(4):
                    ssq = ssq_ps.tile([HL, 512], F32, tag="ssq")
                    nc.tensor.matmul(ssq, hmask_sb, qsqs[t],
                                     start=True, stop=True)
                    nc.vector.tensor_copy(
                        ssq_all[:, ssq_off + t * 512:ssq_off + (t + 1) * 512],
                        ssq)

            def rope_apply(dst, dst_off, ssq_off, c_sb, s_sb, tab_off):
                """Broadcast the rsqrt scales + rope in place over
                dst[:, dst_off:+2048]."""
                sl = slice(dst_off, dst_off + N)
                tab = slice(tab_off, tab_off + N)
                # rs broadcast: [2, 2048] -> [128, 2048] via selector matmul
                rsb_sb = rope.tile([P, N], BF16, name="rsb", tag="rsb")
                for t in range(4):
                    cs = slice(t * 512, (t + 1) * 512)
                    rp = rsb_ps.tile([P, 512], F32, tag="rsb_ps")
                    nc.tensor.matmul(
                        rp, hsel_sb,
                        rs_all[:, ssq_off + t * 512:ssq_off + (t + 1) * 512],
                        start=True, stop=True)
                    nc.scalar.activation(
                        out=rsb_sb[:, cs], in_=rp,
                        func=mybir.ActivationFunctionType.Copy)
                # rope over the full row block (in place on dst)
                m1 = rope.tile([P, N], BF16, name="m1", tag="m1")
                nc.vector.tensor_mul(m1, dst[:, sl], c_sb[:, tab])
                # rotate-half across partitions via SBUF->SBUF DMA (engine-free)
                t1r = rope.tile([P, N], BF16, name="t1r", tag="t1r")
                for h in range(HL):
                    b = h * DH
                    nc.sync.dma_start(out=t1r[b:b + 32, :],
                                      in_=dst[b + 32:b + 64, sl])
                    nc.sync.dma_start(out=t1r[b + 32:b + 64, :],
                                      in_=dst[b:b + 32, sl])
                r1 = rope.tile([P, N], BF16, name="r1", tag="r1")
                nc.vector.tensor_mul(r1, t1r, s_sb[:, tab])
                s2 = rope.tile([P, N], BF16, name="s2", tag="t1r")
                nc.vector.tensor_add(s2, m1, r1)
                nc.vector.tensor_mul(dst[:, sl], s2, rsb_sb)

            def vproj_chunks(lo, hi):
                for t in range(lo, hi):
                    src_sb, w_sb = (xT_sb, wv_sb) if t < 4 else (yT_sb, wcv_sb)
                    tt = t % 4
                    ps = proj_ps.tile([P, 512], F32, tag="proj")
                    for f in range(8):
                        nc.tensor.matmul(ps, w_sb[:, f, :],
                                         src_sb[:, tt, f, :],
                                         start=(f == 0), stop=(f == 7))
                    cs = slice(t * 512, (t + 1) * 512)
                    nc.vector.tensor_copy(vT_sb[:, cs], ps)

            # projections, then one batched rsqrt (Ln+Exp = 2 table loads),
            # then transposes + ropes.
            vT_sb = p1big.tile([P, NK], BF16)
            proj_chunks(wq_sb, xT_sb, qTn, 0, 0)
            proj_chunks(wk_sb, xT_sb, kTn, 0, N)
            proj_chunks(wck_sb, yT_sb, kTn, N, 2 * N)
            vproj_chunks(0, 8)
            nc.scalar.activation(out=ssq_all, in_=ssq_all,
                                 func=mybir.ActivationFunctionType.Ln,
                                 bias=eps2)
            rs_all = p1big.tile([HL, 3 * N], BF16)
            nc.scalar.activation(out=rs_all, in_=ssq_all,
                                 func=mybir.ActivationFunctionType.Exp,
                                 scale=-0.5)

            # ropes first, then the V transposes: the 32 transpose matmuls
            # fill the in-order PE queue exactly while the DVE rope chains
            # run, instead of leaving the PE idle before the first QK.
            rope_apply(qTn, 0, 0, cq_sb, sq_sb, 0)
            rope_apply(kTn, 0, N, ckc_sb, cks_sb, 0)
            rope_apply(kTn, N, 2 * N, ckc_sb, cks_sb, N)

            # transpose vT -> v_all natural layout via PE (4 kc-blocks a time)
            for g in range(NK // 512):
                trp = trps.tile([P, 4, P], BF16, tag="trp")
                for i in range(4):
                    nc.tensor.transpose(trp[:, i, :],
                                        vT_sb[:, (4 * g + i) * P:(4 * g + i + 1) * P],
                                        ident_sb)
                sl4 = slice(4 * g, 4 * g + 4)
                nc.vector.tensor_copy(v_all[:, sl4, 0:64], trp[:, :, 0:64])
                nc.vector.tensor_copy(v_all[:, sl4, 65:129], trp[:, :, 64:128])

        # ---------------- phase 2: attention (+ per-half A2A & out-proj) ----
        with tc.tile_pool(name="st_ps", bufs=2, space="PSUM") as st_ps, \
                tc.tile_pool(name="pv_ps", bufs=1, space="PSUM") as pv_ps, \
                tc.tile_pool(name="p2work", bufs=3) as p2work, \
                tc.tile_pool(name="p2small", bufs=2) as p2small, \
                tc.tile_pool(name="p2out", bufs=2) as p2out:
            # wo load deferred to here: startup DMA bandwidth goes to x/y.
            wo_sb = p2out.tile([P, 8, D], BF16, tag="wo_sb", bufs=1)
            nc.sync.dma_start(out=wo_sb, in_=wo[:, :, :])

            def outproj(qh):
                """Out-projection of one q-half (after its A2A landed)."""
                of_sb = p2out.tile([P, NCORES, P], BF16, name="of_sb",
                                   tag="of")
                for j in range(NCORES):
                    nc.sync.dma_start(out=of_sb[:, j, :],
                                      in_=a2a_out[qh, j, :, :])
                for nn in range(2):  # 2 output col chunks of 512
                    zp = st_ps.tile([P, 512], F32, name="zp", tag="st",
                                    padded_shape=[P, 1024])
                    for j in range(NCORES):
                        nc.tensor.matmul(zp, of_sb[:, j, :],
                                         wo_sb[:, j, nn * 512:(nn + 1) * 512],
                                         start=(j == 0), stop=(j == NCORES - 1))
                    zs = p2out.tile([P, 512], F32, tag="zs")
                    nc.vector.tensor_add(zs, zp,
                                         bo_b[:, nn * 512:(nn + 1) * 512])
                    nc.sync.dma_start(out=out_ext[qh * P:(qh + 1) * P,
                                                  nn * 512:(nn + 1) * 512],
                                      in_=zs)

            for qh in range(2):          # q halves of 1024
                qsl = slice(qh * 1024, (qh + 1) * 1024)
                oT = p2work.tile([P, 1024], BF16, name=f"oT{qh}", tag="oT",
                                 bufs=2)
                pv = [pv_ps.tile([65, 1024], F32, name=f"pv{h}", tag=f"pv{h}",
                                 padded_shape=[P, 1024])
                      for h in range(HL)]
                def emit_pv(kc, es):
                    for h in range(HL):
                        for c in range(2):
                            nc.tensor.matmul(
                                pv[h][:, c * 512:(c + 1) * 512],
                                v_all[:, kc, h * 65:(h + 1) * 65],
                                es[h][:, c * 512:(c + 1) * 512],
                                start=(kc == 0), stop=(kc == NK // P - 1))

                es_prev = None
                for kc in range(NK // P):
                    # 4 QK matmuls emitted adjacently: the h0/h1 pairs sit on
                    # different PE row groups and execute concurrently.
                    sts = [st_ps.tile([P, 1024], F32, name="st", tag="st")
                           for _ in range(HL)]
                    for c in range(2):
                        for h in range(HL):
                            hs = slice(h * DH, (h + 1) * DH)
                            nc.tensor.matmul(
                                sts[h][:, c * 512:(c + 1) * 512],
                                kTn[hs, kc * P:(kc + 1) * P],
                                qTn[hs, qh * 1024 + c * 512: qh * 1024 + (c + 1) * 512],
                                start=True, stop=True)
                    # previous kc's PV fills the PE while this kc's exps run
                    if es_prev is not None:
                        emit_pv(kc - 1, es_prev)
                    es = []
                    for h in range(HL):
                        e = p2work.tile([P, 1024], BF16, name="es", tag="es",
                                        bufs=6)
                        if h == 0:
                            nc.scalar.activation(
                                out=e, in_=sts[h],
                                func=mybir.ActivationFunctionType.Exp,
                                scale=SCALE)
                        else:
                            # Schraudolph bf16 exp on the DVE
                            nc.vector.tensor_scalar(
                                out=e.bitcast(I16), in0=sts[h],
                                scalar1=EXA, scalar2=EXB,
                                op0=mybir.AluOpType.mult,
                                op1=mybir.AluOpType.add)
                        es.append(e)
                    es_prev = es
                emit_pv(NK // P - 1, es_prev)
                # normalize: recip via ACT Ln+Exp(-x), broadcast via PE.
                # Ln/Ln then Exp/Exp: 2 ACT table switches instead of 4.
                lnds = []
                for h in range(HL):
                    lnd = p2small.tile([1, 1024], F32, tag="lnd")
                    nc.scalar.activation(out=lnd, in_=pv[h][64:65, :],
                                         func=mybir.ActivationFunctionType.Ln)
                    lnds.append(lnd)
                for h in range(HL):
                    rdc = p2small.tile([1, 1024], BF16, tag="rdc")
                    nc.scalar.activation(out=rdc, in_=lnds[h],
                                         func=mybir.ActivationFunctionType.Exp,
                                         scale=-1.0)
                    rdb = p2small.tile([DH, 1024], BF16, tag="rdb")
                    nc.gpsimd.partition_broadcast(rdb[0:DH, :], rdc[0:1, :],
                                                  channels=DH)
                    nc.vector.tensor_mul(oT[h * DH:(h + 1) * DH, :],
                                         pv[h][0:64, :], rdb)
                # A2A for this q-half: shard j = 128 positions for dest core j.
                # Core j ends up owning rows {j*128..}+{1024+j*128..}.
                for j in range(NCORES):
                    nc.sync.dma_start(
                        out=a2a_in[qh, j, :, :],
                        in_=oT[:, j * P:(j + 1) * P])
                nc.gpsimd.collective_compute(
                    "AllToAll", mybir.AluOpType.bypass,
                    replica_groups=[list(range(NCORES))],
                    ins=[a2a_in[qh]],
                    outs=[a2a_out[qh]],
                )
            # out-projections emitted last: qh0's executes during qh1's
            # A2A (its own A2A finished long ago); only qh1's is exposed.
            outproj(0)
            outproj(1)
    return nc


def _bf16(a):
    return np.ascontiguousarray(a).astype(ml_dtypes.bfloat16)


def _rope_tables(npos, pos0, g_first, g_second, n_first):
    """Tables [128, npos] for transposed-layout rope with g folded in.

    Row j (within a head, duplicated for 2 local heads):
      out[j] = t[j]*C[j] + t[sigma(j)]*S[j]
      j <  32: C[j]=g[j]*cos[n,j],     S[j]=-g[j+32]*sin[n,j]
      j >= 32: C[j]=g[j]*cos[n,j-32],  S[j]=+g[j-32]*sin[n,j-32]
    g switches from g_first to g_second at position n_first.
    """
    inv = 1.0 / (ROPE_BASE ** (np.arange(0, DH, 2, dtype=np.float64) / DH))
    pos = np.arange(pos0, pos0 + npos, dtype=np.float64)
    ang = pos[:, None] * inv[None, :]          # [npos, 32]
    cos = np.cos(ang).T                         # [32, npos]
    sin = np.sin(ang).T
    C = np.zeros((DH, npos), np.float64)
    S = np.zeros((DH, npos), np.float64)
    g = np.zeros((DH, npos), np.float64)
    g[:, :n_first] = np.asarray(g_first, np.float64)[:, None]
    if n_first < npos:
        g[:, n_first:] = np.asarray(g_second, np.float64)[:, None]
    C[:32] = cos
    C[32:] = cos
    C *= g
    S[:32] = -sin
    S[32:] = sin
    Srot = np.concatenate([g[32:], g[:32]], axis=0)  # g[sigma(j)]
    S *= Srot
    C2 = np.concatenate([C, C], axis=0)  # duplicate for 2 local heads
    S2 = np.concatenate([S, S], axis=0)
    return _bf16(C2), _bf16(S2)


_NC_CACHE = None


def kernel(x, y, W_qkv, W_ckv, W_out, b_out, g_q, g_k, g_ck, n_heads):
    global LAST_RESULT, _NC_CACHE
    x = np.asarray(x, np.float32)
    y = np.asarray(y, np.float32)
    W_qkv = np.asarray(W_qkv, np.float32)
    W_ckv = np.asarray(W_ckv, np.float32)
    W_out = np.asarray(W_out, np.float32)
    b_out = np.asarray(b_out, np.float32)

    def _prearr_x(a):
        # a [2048 pos, 1024 feat] -> [p, chunk, f, 512]:
        # element (f*128+p, c*512+ns) lands at [p, c, f, ns]
        return _bf16(a.T.reshape(8, P, 4, 512).transpose(1, 2, 0, 3))

    xT = _prearr_x(x[0])
    yT = _prearr_x(y[0])
    Wq, Wk, Wv = (W_qkv[:, i * D:(i + 1) * D] for i in range(3))
    Wck, Wcv = (W_ckv[:, i * D:(i + 1) * D] for i in range(2))

    def _prearr(w):
        # [1024, C] row f*128+p -> [p, f, c]: contiguous per-partition DMAs
        return _bf16(w.reshape(8, P, -1).transpose(1, 0, 2))

    woh = _prearr(W_out)
    boh = _bf16(b_out[None, :])

    cqh, sqh = _rope_tables(N, 0, g_q, g_q, N)
    ckch, cksh = _rope_tables(NK, 0, g_k, g_ck, N)
    hm = np.zeros((P, HL), np.float32)
    for h in range(HL):
        hm[h * DH:(h + 1) * DH, h] = 1.0 / DH
    hmh = _bf16(hm)
    hs = np.zeros((HL, P), np.float32)
    for h in range(HL):
        hs[h, h * DH:(h + 1) * DH] = 1.0
    hsh = _bf16(hs)
    idh = _bf16(np.eye(P, dtype=np.float32))

    in_maps = []
    for c in range(NCORES):
        sl = slice(c * DL, (c + 1) * DL)
        in_maps.append({
            "xT": xT, "yT": yT,
            "wq": _prearr(Wq[:, sl]), "wk": _prearr(Wk[:, sl]),
            "wv": _prearr(Wv[:, sl]), "wck": _prearr(Wck[:, sl]),
            "wcv": _prearr(Wcv[:, sl]),
            "wo": woh, "bo": boh,
            "cq": cqh, "sq": sqh, "ckc": ckch, "cks": cksh,
            "hmask": hmh, "hsel": hsh, "ident": idh,
        })

    if _NC_CACHE is None:
        _NC_CACHE = build_nc()
        if not _NC_CACHE.is_finalized():
            _NC_CACHE.finalize()
    nc = _NC_CACHE

    res = run_bass_kernel_spmd(
        nc, in_maps, core_ids=list(range(NCORES)),
        trace=bool(os.environ.get("BASS_TRACE")),
    )
    LAST_RESULT = res
    out = np.empty((N, D), np.float32)
    for c in range(NCORES):
        o = np.asarray(res.results[c]["out"], np.float32)
        out[c * P:(c + 1) * P] = o[0:P]
        out[N // 2 + c * P:N // 2 + (c + 1) * P] = o[P:2 * P]
    return out[None, :, :]
